# revision 1
# baseline (speedup 1.0000x reference)
"""Trainium2 Bass kernel for nn_Block0 (bilinear-LUT resample + 7x7/7 dwconv
+ LayerNorm + MLP + residual), 8-core SPMD.

- Shard: core h computes output rows [8h, 8h+8) for ALL 4 samples (LUTs are
  batch-shared: each bilinear weight column serves 4 samples x 96 channels).
- Launch 1: per sampled point, 4 bilinear corner weights host-scattered into
  a dense fp16 column over a 128-pixel source slab (8x16 image patch);
  PE matmuls img_slab[128px,(32c,4b)]^T @ W[128px,cols] -> V in PSUM;
  DVE/ACT drain to fp16; DMA V (slab-sorted columns) to DRAM.
- Host: permutes V columns to pixel-major (px, tap) order.
- Launch 2: 49 per-partition-scalar MACs reduce taps -> y; LayerNorm folded
  into pw1 (stats via ones-matmul); exact GELU on ACT; pw2 (+gamma folded).
  Residual add + unshard on host.
"""
from contextlib import ExitStack

import numpy as np

import concourse.bass as bass
import concourse.mybir as mybir
import concourse.tile as tile
import concourse.bacc as bacc
from concourse.bass_utils import run_bass_kernel_spmd

B, C, H, W = 4, 96, 64, 128
UPH, UPW = 448, 896
NCORES = 8
ROWS_PER_CORE = 8
PX = ROWS_PER_CORE * W         # 1024
NSLAB = 81
CB = 384                       # free index c*4+b
WIN = 512
PXW = 128          # stage-2 pixel window
PXWM = 256         # MLP pixel window
F16 = mybir.dt.float16
F32 = mybir.dt.float32
F32R = mybir.dt.float32r
ALU = mybir.AluOpType
ACTF = mybir.ActivationFunctionType


# ----------------------------------------------------------------- host prep
def _point_tables(lut1, lut2):
    p = np.arange(UPH * UPW) // UPW
    q = np.arange(UPH * UPW) % UPW
    lut = np.where((q < 448)[:, None], lut1, lut2)
    cx = lut[:, 0].astype(np.float32)
    cy = lut[:, 1].astype(np.float32)
    x1 = np.clip(np.floor(cx).astype(np.int32), 0, W - 1)
    x2 = np.clip(x1 + 1, 0, W - 1)
    y1 = np.clip(np.floor(cy).astype(np.int32), 0, H - 1)
    y2 = np.clip(y1 + 1, 0, H - 1)
    dx1 = cx - x1.astype(np.float32)
    dx2 = x2.astype(np.float32) - cx
    dy1 = cy - y1.astype(np.float32)
    dy2 = y2.astype(np.float32) - cy
    r0 = np.minimum(y1, H - 2)
    c0 = np.minimum(x1, W - 2)
    g = r0 // 7
    xb = c0 // 15
    cs = np.minimum(15 * xb, W - 16)
    return dict(x1=x1, x2=x2, y1=y1, y2=y2,
                w11=dx2 * dy2, w12=dx1 * dy2, w21=dx2 * dy1, w22=dx1 * dy1,
                g=g, cs=cs, slab=g * 9 + xb)


def _build_stage1_data(x, T):
    # img slab tensor [128, 81, 384] fp16 (shared across cores)
    img_cb = np.transpose(np.asarray(x), (2, 3, 1, 0)).reshape(H, W, CB)
    img_slab = np.zeros((NSLAB, 128, CB), np.float32)
    for g in range(9):
        for xb in range(9):
            cs = min(15 * xb, W - 16)
            img_slab[g * 9 + xb] = img_cb[7 * g:7 * g + 8,
                                          cs:cs + 16, :].reshape(128, CB)
    img_flat = np.ascontiguousarray(
        np.transpose(img_slab, (1, 0, 2))).astype(np.float16)

    per_core = []
    counts = np.zeros((NCORES, NSLAB), np.int64)
    for h in range(NCORES):
        n = np.arange(h * 56 * UPW, (h + 1) * 56 * UPW)
        slabs = T["slab"][n]
        o = np.argsort(slabs, kind="stable")
        per_core.append((n[o], slabs[o]))
        counts[h] = np.bincount(slabs, minlength=NSLAB)

    S = ((counts.max(axis=0) + 127) // 128) * 128
    off = np.zeros(NSLAB + 1, np.int64)
    off[1:] = np.cumsum(S)
    ncols_pad = int(((off[-1] + WIN - 1) // WIN) * WIN)

    pieces = []
    for s in range(NSLAB):
        a, b = int(off[s]), int(off[s] + S[s])
        while a < b:
            e = min(b, (a // WIN + 1) * WIN)
            pieces.append((s, a, e))
            a = e

    cores = []
    i_px = np.arange(PX) // W
    j_px = np.arange(PX) % W
    u_t = np.arange(49) // 7
    v_t = np.arange(49) % 7
    for h in range(NCORES):
        order_n, slab_sorted = per_core[h]
        cnt = np.bincount(slab_sorted, minlength=NSLAB)
        first = np.concatenate([[0], np.cumsum(cnt)[:-1]])
        pos = np.arange(len(order_n)) - first[slab_sorted] + off[slab_sorted]

        Wf = np.zeros((ncols_pad, 128), np.float32)
        n = order_n
        g, cs = T["g"][n], T["cs"][n]
        for (yy, xx, ww) in ((T["y1"], T["x1"], T["w11"]),
                             (T["y1"], T["x2"], T["w12"]),
                             (T["y2"], T["x1"], T["w21"]),
                             (T["y2"], T["x2"], T["w22"])):
            krow = (yy[n] - 7 * g) * 16 + (xx[n] - cs)
            np.add.at(Wf, (pos, krow), ww[n])
        Wmat = np.ascontiguousarray(Wf.T).astype(np.float16)

        nn = ((7 * (8 * h + i_px[:, None]) + u_t[None, :]) * UPW
              + 7 * j_px[:, None] + v_t[None, :]).reshape(-1)
        n2pos = np.zeros(UPH * UPW, np.int64)
        n2pos[order_n] = pos
        cores.append(dict(Wmat=Wmat, perm=n2pos[nn]))
    return img_flat, cores, pieces, ncols_pad


# ------------------------------------------------------------- device progs
def _build_launch1(ncols_pad, pieces):
    nc = bacc.Bacc("TRN2", target_bir_lowering=False, num_devices=NCORES)
    img_d = nc.dram_tensor("img", [128, NSLAB, CB], F16, kind="ExternalInput").ap()
    w_d = nc.dram_tensor("wmat", [128, ncols_pad], F16, kind="ExternalInput").ap()
    v_d = nc.dram_tensor("vout", [128, 3, ncols_pad], F16, kind="ExternalOutput").ap()

    nwin = ncols_pad // WIN
    bywin = [[] for _ in range(nwin)]
    for (s, a, b) in pieces:
        bywin[a // WIN].append((s, a, b))

    with tile.TileContext(nc) as tc, ExitStack() as ctx:
        const = ctx.enter_context(tc.tile_pool(name="const", bufs=1))
        wpool = ctx.enter_context(tc.tile_pool(name="wpool", bufs=3))
        spool = ctx.enter_context(tc.tile_pool(name="spool", bufs=3))
        psum = ctx.enter_context(tc.tile_pool(name="psum", bufs=2, space="PSUM"))

        img_t = const.tile([128, NSLAB, CB], F16)
        nc.sync.dma_start(out=img_t, in_=img_d)

        for wi in range(nwin):
            w_t = wpool.tile([128, WIN], F16)
            nc.sync.dma_start(out=w_t, in_=w_d[:, wi * WIN:(wi + 1) * WIN])
            st = spool.tile([128, 3, WIN], F16)
            for j in range(3):
                ps = psum.tile([128, WIN], F32, tag=f"ps{j}")
                for (s, a, b) in bywin[wi]:
                    al, bl = a - wi * WIN, b - wi * WIN
                    nc.tensor.matmul(
                        ps[:, al:bl],
                        img_t[:, s, 128 * j:128 * (j + 1)],
                        w_t[:, al:bl],
                        start=True, stop=True)
                if (wi + j) % 2 == 0:
                    nc.vector.tensor_copy(out=st[:, j, :], in_=ps[:, :])
                else:
                    nc.scalar.copy(out=st[:, j, :], in_=ps[:, :])
            nc.sync.dma_start(out=v_d[:, :, wi * WIN:(wi + 1) * WIN], in_=st)
    nc.compile()
    return nc


def _build_launch2():
    nc = bacc.Bacc("TRN2", target_bir_lowering=False, num_devices=NCORES)
    vij_d = nc.dram_tensor("vij", [128, 3, PX, 49], F16, kind="ExternalInput").ap()
    dwt_d = nc.dram_tensor("dwt", [128, 3, 49], F32, kind="ExternalInput").ap()
    dwb_d = nc.dram_tensor("dwb", [128, 3], F32, kind="ExternalInput").ap()
    ones_d = nc.dram_tensor("onesb", [128, 4], F32, kind="ExternalInput").ap()
    s1_d = nc.dram_tensor("s1t", [128, 12], F32, kind="ExternalInput").ap()   # NEGATED s1
    c1_d = nc.dram_tensor("c1t", [128, 12], F32, kind="ExternalInput").ap()
    pw1_d = nc.dram_tensor("pw1bd", [128, 3, 12, 128], F32R, kind="ExternalInput").ap()
    pw2_d = nc.dram_tensor("pw2bd", [128, 12, 3, 128], F32R, kind="ExternalInput").ap()
    b2_d = nc.dram_tensor("b2t", [128, 3], F32, kind="ExternalInput").ap()
    br_d = nc.dram_tensor("branch", [128, 3, PX], F32, kind="ExternalOutput").ap()
    mu_d = nc.dram_tensor("mu_scratch", [4, PX], F32)
    rs_d = nc.dram_tensor("rs_scratch", [4, PX], F32)

    nwin = PX // PXW
    with tile.TileContext(nc) as tc, ExitStack() as ctx:
        const = ctx.enter_context(tc.tile_pool(name="const", bufs=1))
        vpool = ctx.enter_context(tc.tile_pool(name="vpool", bufs=2))
        work = ctx.enter_context(tc.tile_pool(name="work", bufs=2))
        psum = ctx.enter_context(tc.tile_pool(name="psum", bufs=2, space="PSUM"))

        dwt = const.tile([128, 3, 49], F32)
        dwb = const.tile([128, 3], F32)
        onesb = const.tile([128, 4], F32)
        s1t = const.tile([128, 12], F32)
        c1t = const.tile([128, 12], F32)
        pw1 = const.tile([128, 3, 12, 128], F32R)
        pw2 = const.tile([128, 12, 3, 128], F32R)
        b2t = const.tile([128, 3], F32)
        for t, d in ((dwt, dwt_d), (dwb, dwb_d), (onesb, ones_d), (s1t, s1_d),
                     (c1t, c1_d), (pw1, pw1_d), (pw2, pw2_d), (b2t, b2_d)):
            nc.sync.dma_start(out=t, in_=d)

        y = const.tile([128, 3, PX], F32R, tag="yacc")
        for wi in range(nwin):
            vt = vpool.tile([128, 3, PXW, 49], F16)
            nc.sync.dma_start(out=vt, in_=vij_d[:, :, wi * PXW:(wi + 1) * PXW, :])
            for j in range(3):
                ysl = y[:, j, wi * PXW:(wi + 1) * PXW]
                nc.vector.tensor_scalar(ysl, vt[:, j, :, 0],
                                        dwt[:, j, 0:1], None, ALU.mult)
                for k in range(1, 49):
                    nc.vector.scalar_tensor_tensor(
                        out=ysl, in0=vt[:, j, :, k], scalar=dwt[:, j, k:k + 1],
                        in1=ysl, op0=ALU.mult, op1=ALU.add)
                nc.vector.tensor_scalar(ysl, ysl, dwb[:, j:j + 1], None, ALU.add)

        # LN stats
        ysq = const.tile([128, 3, PX], F32, tag="ysq")
        for j in range(3):
            nc.vector.tensor_mul(ysq[:, j, :], y[:, j, :].bitcast(F32), y[:, j, :].bitcast(F32))
        mu = const.tile([4, PX], F32, tag="muv")
        rstd = const.tile([4, PX], F32, tag="rstdv")
        for half in range(2):
            sl = slice(half * 512, (half + 1) * 512)
            mu_ps = psum.tile([4, 512], F32, tag="mups")
            m2_ps = psum.tile([4, 512], F32, tag="m2ps")
            for j in range(3):
                nc.tensor.matmul(mu_ps[:, :], onesb[:, :],
                                 y[:, j, sl].bitcast(F32),
                                 start=(j == 0), stop=(j == 2))
            for j in range(3):
                nc.tensor.matmul(m2_ps[:, :], onesb[:, :],
                                 ysq[:, j, sl],
                                 start=(j == 0), stop=(j == 2))
            t4 = work.tile([4, 512], F32, tag="t4")
            nc.vector.tensor_copy(out=mu[:, sl], in_=mu_ps[:, :])
            nc.vector.tensor_mul(t4, mu[:, sl], mu[:, sl])
            nc.vector.tensor_sub(t4, m2_ps[:, :], t4)
            nc.vector.tensor_scalar(t4, t4, 1e-6, None, ALU.add)
            nc.scalar.activation(out=t4, in_=t4, func=ACTF.Sqrt)
            nc.vector.reciprocal(out=rstd[:, sl], in_=t4)

        # broadcast mu/rstd to 128 partitions via DRAM bounce
        nc.sync.dma_start(out=mu_d.ap(), in_=mu)
        nc.sync.dma_start(out=rs_d.ap(), in_=rstd)
        mub = const.tile([128, PX], F32, tag="mub")
        rstdb = const.tile([128, PX], F32, tag="rstdb")
        mu_ap = mu_d.ap()
        rs_ap = rs_d.ap()
        mu_bc = bass.AP(tensor=mu_ap.tensor, offset=mu_ap.offset,
                        ap=[[0, 32]] + list(mu_ap.ap))
        rs_bc = bass.AP(tensor=rs_ap.tensor, offset=rs_ap.offset,
                        ap=[[0, 32]] + list(rs_ap.ap))
        nc.sync.dma_start(out=mub, in_=mu_bc)
        nc.sync.dma_start(out=rstdb, in_=rs_bc)

        # MLP
        for wi in range(PX // PXWM):
            sl = slice(wi * PXWM, (wi + 1) * PXWM)
            h_t = work.tile([128, 12, PXWM], F32R, tag="hti")
            for mi in range(12):
                zp = psum.tile([128, PXWM], F32, tag="zps")
                for j in range(3):
                    nc.tensor.matmul(zp[:, :], pw1[:, j, mi, :],
                                     y[:, j, sl],
                                     start=(j == 0), stop=(j == 2))
                t1 = work.tile([128, PXWM], F32, tag="t1")
                # t1 = mub*(-s1) + z
                nc.vector.scalar_tensor_tensor(
                    out=t1, in0=mub[:, sl], scalar=s1t[:, mi:mi + 1],
                    in1=zp[:, :], op0=ALU.mult, op1=ALU.add)
                nc.vector.tensor_mul(t1, t1, rstdb[:, sl])
                nc.vector.tensor_scalar(t1, t1, c1t[:, mi:mi + 1], None, ALU.add)
                nc.scalar.activation(out=h_t[:, mi, :], in_=t1, func=ACTF.Gelu)
            for mj in range(3):
                op = psum.tile([128, PXWM], F32, tag="ops")
                for ki in range(12):
                    nc.tensor.matmul(op[:, :], pw2[:, ki, mj, :],
                                     h_t[:, ki, :],
                                     start=(ki == 0), stop=(ki == 11))
                ot = work.tile([128, PXWM], F32, tag="ot")
                nc.vector.tensor_scalar(ot, op[:, :], b2t[:, mj:mj + 1],
                                        None, ALU.add)
                nc.sync.dma_start(out=br_d[:, mj, sl], in_=ot)
    nc.compile()
    return nc


def _blockdiag(blk):
    """blk [32 out_sub, 32 in_sub] -> lhsT [(in,4b), (out,4b)] 128x128."""
    t = np.zeros((128, 128), np.float32)
    idx = np.arange(32) * 4
    for b in range(4):
        t[np.ix_(idx + b, idx + b)] = blk.T
    return t


# ------------------------------------------------------------------ kernel()
_CACHE = {}


def kernel(x, lut1, lut2, dw_w, dw_b, norm_w, norm_b, pw1_w, pw1_b, pw2_w,
           pw2_b, gamma):
    x = np.asarray(x, np.float32)
    lut1 = np.asarray(lut1, np.float32)
    lut2 = np.asarray(lut2, np.float32)
    dw_w2 = np.asarray(dw_w, np.float32).reshape(C, 49)
    dw_b = np.asarray(dw_b, np.float32)
    norm_w = np.asarray(norm_w, np.float32)
    norm_b = np.asarray(norm_b, np.float32)
    pw1_w = np.asarray(pw1_w, np.float32)
    pw1_b = np.asarray(pw1_b, np.float32)
    pw2_w = np.asarray(pw2_w, np.float32)
    pw2_b = np.asarray(pw2_b, np.float32)
    gamma = np.asarray(gamma, np.float32)

    T = _point_tables(lut1, lut2)
    img_flat, cores, pieces, ncols_pad = _build_stage1_data(x, T)

    key1 = ("l1", ncols_pad, tuple(pieces))
    if key1 not in _CACHE:
        _CACHE.clear()
        _CACHE[key1] = _build_launch1(ncols_pad, pieces)
        _CACHE["l2"] = _build_launch2()
    nc1 = _CACHE[key1]
    nc2 = _CACHE["l2"]

    maps1 = [{"img": img_flat, "wmat": cores[h]["Wmat"]} for h in range(NCORES)]
    res1 = run_bass_kernel_spmd(nc1, maps1, list(range(NCORES)))

    vij = [np.ascontiguousarray(
        res1.results[h]["vout"][:, :, cores[h]["perm"]]
        .reshape(128, 3, PX, 49)) for h in range(NCORES)]

    cidx = np.arange(128) // 4
    bidx = np.arange(128) % 4
    dwt = np.zeros((128, 3, 49), np.float32)
    dwb = np.zeros((128, 3), np.float32)
    for j in range(3):
        dwt[:, j, :] = dw_w2[32 * j + cidx, :]
        dwb[:, j] = dw_b[32 * j + cidx]
    onesb = np.zeros((128, 4), np.float32)
    onesb[np.arange(128), bidx] = 1.0 / C

    pw1g = pw1_w * norm_w[None, :]
    s1 = pw1g.sum(axis=1)
    c1 = pw1_w @ norm_b + pw1_b
    pw2g = gamma[:, None] * pw2_w
    b2g = gamma * pw2_b
    s1t = np.zeros((128, 12), np.float32)
    c1t = np.zeros((128, 12), np.float32)
    for mi in range(12):
        s1t[:, mi] = -s1[32 * mi + cidx]      # negated for the MAC form
        c1t[:, mi] = c1[32 * mi + cidx]
    pw1bd = np.zeros((128, 3, 12, 128), np.float32)
    pw2bd = np.zeros((128, 12, 3, 128), np.float32)
    for kj in range(3):
        for mi in range(12):
            pw1bd[:, kj, mi, :] = _blockdiag(
                pw1g[32 * mi:32 * mi + 32, 32 * kj:32 * kj + 32])
    for ki in range(12):
        for mj in range(3):
            pw2bd[:, ki, mj, :] = _blockdiag(
                pw2g[32 * mj:32 * mj + 32, 32 * ki:32 * ki + 32])
    b2t = np.zeros((128, 3), np.float32)
    for mj in range(3):
        b2t[:, mj] = b2g[32 * mj + cidx]

    maps2 = [{"vij": vij[h], "dwt": dwt, "dwb": dwb, "onesb": onesb,
              "s1t": s1t, "c1t": c1t, "pw1bd": pw1bd, "pw2bd": pw2bd,
              "b2t": b2t} for h in range(NCORES)]
    res2 = run_bass_kernel_spmd(nc2, maps2, list(range(NCORES)))

    out = np.array(x, np.float32, copy=True)
    for h in range(NCORES):
        br4 = res2.results[h]["branch"].reshape(32, 4, 3, ROWS_PER_CORE, W)
        for j in range(3):
            out[:, 32 * j:32 * j + 32, 8 * h:8 * h + 8, :] += np.transpose(
                br4[:, :, j], (1, 0, 2, 3))
    return out

